# revision 9
# baseline (speedup 1.0000x reference)
"""Trainium2 Bass kernel: ColBERT-like scorer.

Computes s[b,o] = LOGIT_SCALE/sqrt(Lq*Lk) * sum_i logsumexp_j(ALPHA * qn[b,i]@kn[o,j]) / ALPHA
with k_mask masking (masked j excluded), q/k L2-normalized along D.

Sharding: O=128 docs split across 8 cores (16 docs each); fully parallel,
concat along axis 1 at the end.

Per-core algorithm (all on device except mask bookkeeping):
  - sumsq via GPSIMD square + DVE tensor_scalar accum; rsqrt(x) =
    exp(-0.5*ln(x)) on ACT (single natural_log_exp table set, forced via an
    explicit LoadActFuncSet so walrus never thrashes table loads).
  - q stays RAW; its norm scale is folded into the exp: per-partition
    activation scale = ALPHA * rsqrt(|q_i|^2).
  - k normalization + mask fold into the PE transpose: kT_n = k_tile.T @
    diag(valid * rsqrt(|k_j|^2)), diag built on GPSIMD from a host identity.
  - main loop: float32r matmul S_raw=[Lq, 2*Lk] into PSUM, ACT
    exp(arq_i * S_raw) over [128,1024] (bf16 out into a per-b [128,16,256]
    tile), then a bf16 pairwise-add tree on DVE (2x mode) + one
    tensor_reduce into sums.
  - subtract n_masked, Ln, ones-matmul to reduce over Lq, final scale.
"""

import os

import numpy as np

import concourse.bacc as bacc
import concourse.bass as bass
import concourse.tile as tile
from concourse import mybir
from concourse.bass_utils import run_bass_kernel_spmd

B, Lq, D = 16, 128, 128
O_FULL, Lk = 128, 256
N_CORES = 8
O_SH = O_FULL // N_CORES          # 16 docs per core
NPAIR = O_SH // 2                 # 8 doc pairs per core
ALPHA = 12.0
LOGIT_SCALE = float(np.exp(-np.log(0.07)))
FINAL_SCALE = LOGIT_SCALE / ALPHA / (float(Lq * Lk) ** 0.5 + 1e-6)
LN_ALPHA = float(np.log(ALPHA))

F32 = mybir.dt.float32
BF16 = mybir.dt.bfloat16
_MM_DT_NAME = os.environ.get("COLBERT_MM_DT", "float32r")
MM_DT = getattr(mybir.dt, _MM_DT_NAME)
# natural_log_exp_and_others in act_info.json — holds both Exp and Ln
ACT_SET_LN_EXP = 6

AF = mybir.ActivationFunctionType
ALU = mybir.AluOpType


def _emit_kernel(tc, q_d, k_d, valid_d, nmb_d, ident_d, y_d):
    nc = tc.nc

    with (
        tc.tile_pool(name="const", bufs=1) as constp,
        tc.tile_pool(name="raw", bufs=1) as rawp,
        tc.tile_pool(name="kn", bufs=NPAIR) as knp,
        tc.tile_pool(name="qn", bufs=4) as qnp,
        tc.tile_pool(name="stat", bufs=1) as statp,
        tc.tile_pool(name="ss", bufs=4) as ssp,
        tc.tile_pool(name="scr", bufs=2) as scrp,
        tc.tile_pool(name="diag", bufs=4) as diagp,
        tc.tile_pool(name="exp", bufs=2) as expp,
        tc.tile_pool(name="ps_prep", bufs=2, space="PSUM") as psprep,
        tc.tile_pool(name="ps_main", bufs=2, space="PSUM") as psmain,
        tc.tile_pool(name="ps_fin", bufs=1, space="PSUM") as psfin,
    ):
        # Pin the ACT table set that contains BOTH Exp and Ln so walrus
        # doesn't alternate between exp_and_others / natural_log sets.
        nc.scalar.add_instruction(
            mybir.InstLoadActFuncSet(
                name=nc.get_next_instruction_name(),
                act_func_set_id=ACT_SET_LN_EXP,
            )
        )

        # ---- constants from host ----
        ident = constp.tile([128, 128], F32, tag="ident", name="ident")
        nc.sync.dma_start(ident[:], ident_d)
        valid = constp.tile([128, 2 * O_SH], F32, tag="valid", name="valid")
        nc.sync.dma_start(valid[:], valid_d)
        nmb = constp.tile([128, B * O_SH], F32, tag="nmb", name="nmb")
        nc.sync.dma_start(nmb[:], nmb_d)
        ones = constp.tile([128, 1], F32, tag="ones", name="ones")
        nc.vector.memset(ones[:], 1.0)
        lnalpha = constp.tile([128, 1], F32, tag="lna", name="lnalpha")
        nc.vector.memset(lnalpha[:], LN_ALPHA)

        # ---- raw input tiles ----
        # kraw[p, t, d] where t = o*2 + h, j = h*128 + p
        kraw = rawp.tile([128, 2 * O_SH, D], F32, tag="kraw", name="kraw")
        k_re = k_d.rearrange("o (h p) d -> p (o h) d", h=2, p=128)
        # qraw[p, b, d] where p = i
        qraw = rawp.tile([128, B, D], F32, tag="qraw", name="qraw")
        q_re = q_d.rearrange("b i d -> i b d")

        sums = statp.tile([128, B * O_SH], F32, tag="sums", name="sums")
        arq = statp.tile([128, B], F32, tag="arq", name="arq")  # ALPHA*rsqrt(|q|^2)

        knT = [None] * NPAIR   # [d, (o2 h p)] = [128, 512] per pair, MM_DT
        qnT = [None] * 4       # [d, (b4 i)] = [128, 512] per group of 4 b, MM_DT

        def prep_q_group(g):
            """Raw-transpose q[b] for b in [4g, 4g+4); alpha*rsqrt norms -> arq."""
            sl = slice(g * 4, (g + 1) * 4)
            nc.sync.dma_start(qraw[:, sl, :], q_re[:, sl, :])
            sq = scrp.tile([128, 4, D], F32, tag="scr", name=f"qsq{g}")
            nc.gpsimd.tensor_tensor(
                out=sq[:], in0=qraw[:, sl, :], in1=qraw[:, sl, :], op=ALU.mult
            )
            ss = ssp.tile([128, 4], F32, tag="ss", name=f"ssq{g}")
            for t in range(4):
                nc.vector.tensor_scalar(
                    out=sq[:, t, :],
                    in0=sq[:, t, :],
                    scalar1=1.0,
                    scalar2=None,
                    op0=ALU.mult,
                    op1=ALU.add,
                    accum_out=ss[:, t : t + 1],
                )
            # arq = ALPHA * ss^-0.5 = exp(-0.5*ln(ss) + ln(ALPHA))
            nc.scalar.activation(ss[:], ss[:], AF.Ln)
            nc.scalar.activation(
                arq[:, sl], ss[:], AF.Exp, scale=-0.5, bias=lnalpha[:]
            )
            ps = psprep.tile([128, 512], F32, tag="tp", name=f"qtp{g}")
            for t in range(4):
                nc.tensor.matmul(
                    ps[:, t * 128 : (t + 1) * 128],
                    qraw[:, g * 4 + t, :],
                    ident[:],
                    is_transpose=True,
                    start=(t == 0),
                    stop=(t == 3),
                )
            sb = qnp.tile([128, 512], MM_DT, tag="qnT", name=f"qnT{g}")
            nc.vector.tensor_copy(sb[:], ps[:])
            qnT[g] = sb

        def prep_k_pair(p):
            """Fused normalize+mask+transpose of k docs 2p, 2p+1."""
            tsl = slice(p * 4, (p + 1) * 4)  # 4 half-tiles
            nc.sync.dma_start(kraw[:, tsl, :], k_re[:, tsl, :])
            sq = scrp.tile([128, 4, D], F32, tag="scr", name=f"ksq{p}")
            nc.gpsimd.tensor_tensor(
                out=sq[:], in0=kraw[:, tsl, :], in1=kraw[:, tsl, :], op=ALU.mult
            )
            ss = ssp.tile([128, 4], F32, tag="ss", name=f"ssk{p}")
            for t in range(4):
                nc.vector.tensor_scalar(
                    out=sq[:, t, :],
                    in0=sq[:, t, :],
                    scalar1=1.0,
                    scalar2=None,
                    op0=ALU.mult,
                    op1=ALU.add,
                    accum_out=ss[:, t : t + 1],
                )
            # rk = ss^-0.5, then fold the k mask in (masked row -> 0)
            nc.scalar.activation(ss[:], ss[:], AF.Ln)
            nc.scalar.activation(ss[:], ss[:], AF.Exp, scale=-0.5)
            nc.vector.tensor_tensor(
                out=ss[:], in0=ss[:], in1=valid[:, tsl], op=ALU.mult
            )
            ps = psprep.tile([128, 512], F32, tag="tp", name=f"ktp{p}")
            for t in range(4):
                dg = diagp.tile([128, 128], F32, tag="dg", name=f"dg{p}_{t}")
                nc.gpsimd.tensor_scalar(
                    out=dg[:],
                    in0=ident[:],
                    scalar1=ss[:, t : t + 1],
                    scalar2=None,
                    op0=ALU.mult,
                )
                # kT_n quarter = k_tile.T @ diag(r_masked)  (fp32 matmul)
                nc.tensor.matmul(
                    ps[:, t * 128 : (t + 1) * 128],
                    kraw[:, p * 4 + t, :],
                    dg[:],
                    start=(t == 0),
                    stop=(t == 3),
                )
            sb = knp.tile([128, 512], MM_DT, tag="knT", name=f"knT{p}")
            nc.vector.tensor_copy(sb[:], ps[:])
            knT[p] = sb

        # Emit prep: all of k (main loop needs every pair), q interleaved.
        prep_q_group(0)
        for p in range(NPAIR):
            prep_k_pair(p)
            if p % 2 == 1 and p // 2 + 1 < 4:
                prep_q_group(p // 2 + 1)

        # ---- main loop: per b, 4 psum groups of 2 doc-pairs ----
        for b in range(B):
            et = expp.tile([128, O_SH, 2 * Lk // 2], BF16, tag="E", name=f"E{b}")
            # et[p, doc, j'] with j' in [0,256); exp written per group of 4 docs
            for g in range(4):
                ps = psmain.tile([128, 1024], F32, tag="S", name=f"S{b}_{g}")
                for u in range(2):
                    nc.tensor.matmul(
                        ps[:, u * 512 : (u + 1) * 512],
                        qnT[b // 4][:, (b % 4) * 128 : (b % 4 + 1) * 128],
                        knT[g * 2 + u][:],
                        start=True,
                        stop=True,
                    )
                nc.scalar.activation(
                    et[:, g * 4 : (g + 1) * 4, :].rearrange("p a c -> p (a c)"),
                    ps[:],
                    AF.Exp,
                    scale=arq[:, b : b + 1],
                )
            # pairwise-add tree over j' (bf16, 2x mode), then one reduce
            nc.vector.tensor_tensor(
                out=et[:, :, 0:128], in0=et[:, :, 0:128], in1=et[:, :, 128:256],
                op=ALU.add,
            )
            nc.vector.tensor_tensor(
                out=et[:, :, 0:64], in0=et[:, :, 0:64], in1=et[:, :, 64:128],
                op=ALU.add,
            )
            nc.vector.tensor_tensor(
                out=et[:, :, 0:32], in0=et[:, :, 0:32], in1=et[:, :, 32:64],
                op=ALU.add,
            )
            nc.vector.tensor_reduce(
                out=sums[:, b * O_SH : (b + 1) * O_SH],
                in_=et[:, :, 0:32],
                op=ALU.add,
                axis=mybir.AxisListType.X,
            )

        # ---- final: lse = ln(sum - n_masked); s = scale * sum_i lse ----
        sub = statp.tile([128, B * O_SH], F32, tag="sub", name="sub")
        nc.vector.tensor_tensor(out=sub[:], in0=sums[:], in1=nmb[:], op=ALU.subtract)
        nc.scalar.activation(sub[:], sub[:], AF.Ln)
        fin = psfin.tile([1, B * O_SH], F32, tag="fin", name="fin")
        nc.tensor.matmul(fin[:], ones[:], sub[:], start=True, stop=True)
        out_sb = statp.tile([1, B * O_SH], F32, tag="out", name="out_sb")
        nc.scalar.mul(out_sb[:], fin[:], FINAL_SCALE)
        nc.sync.dma_start(y_d, out_sb[:])


def build_nc():
    nc = bacc.Bacc("TRN2", target_bir_lowering=False, debug=False)
    q_d = nc.dram_tensor("q", [B, Lq, D], F32, kind="ExternalInput")
    k_d = nc.dram_tensor("k", [O_SH, Lk, D], F32, kind="ExternalInput")
    valid_d = nc.dram_tensor("valid", [128, 2 * O_SH], F32, kind="ExternalInput")
    nmb_d = nc.dram_tensor("nmb", [128, B * O_SH], F32, kind="ExternalInput")
    ident_d = nc.dram_tensor("ident", [128, 128], F32, kind="ExternalInput")
    y_d = nc.dram_tensor("y", [1, B * O_SH], F32, kind="ExternalOutput")
    with tile.TileContext(nc) as tc:
        _emit_kernel(
            tc, q_d.ap(), k_d.ap(), valid_d.ap(), nmb_d.ap(), ident_d.ap(), y_d.ap()
        )
    nc.compile()
    return nc


_NC = None


def _get_nc():
    global _NC
    if _NC is None:
        _NC = build_nc()
    return _NC


def make_in_maps(q, k, q_mask, k_mask):
    q = np.ascontiguousarray(np.asarray(q, dtype=np.float32))
    k = np.asarray(k, dtype=np.float32)
    k_mask = np.asarray(k_mask, dtype=bool)
    kvalid = (~k_mask).astype(np.float32)               # [O, Lk]
    nmask = k_mask.sum(axis=1).astype(np.float32)       # [O]
    ident = np.eye(128, dtype=np.float32)
    in_maps = []
    for c in range(N_CORES):
        osl = slice(c * O_SH, (c + 1) * O_SH)
        # valid[p, o*2+h] matching kraw tile order
        valid_t = np.ascontiguousarray(
            kvalid[osl].reshape(O_SH, 2, 128).transpose(2, 0, 1).reshape(128, 2 * O_SH)
        )
        nmb_row = np.tile(nmask[osl], B)                # col = b*16 + o
        nmb_t = np.ascontiguousarray(
            np.broadcast_to(nmb_row[None, :], (128, B * O_SH))
        )
        in_maps.append(
            {
                "q": q,
                "k": np.ascontiguousarray(k[osl]),
                "valid": valid_t,
                "nmb": nmb_t,
                "ident": ident,
            }
        )
    return in_maps


def _postprocess(results, q_mask, k_mask):
    y = np.concatenate(
        [results[c]["y"].reshape(B, O_SH) for c in range(N_CORES)], axis=1
    ).astype(np.float32)
    q_mask = np.asarray(q_mask, dtype=bool)
    k_mask = np.asarray(k_mask, dtype=bool)
    # exact emulation of the reference's isfinite -> 0 replacement
    y[q_mask.any(axis=1), :] = 0.0
    y[:, k_mask.all(axis=1)] = 0.0
    return y


def run(q, k, q_mask, k_mask, trace=False, **kwargs):
    nc = _get_nc()
    in_maps = make_in_maps(q, k, q_mask, k_mask)
    res = run_bass_kernel_spmd(
        nc, in_maps, core_ids=list(range(N_CORES)), trace=trace, **kwargs
    )
    return _postprocess(res.results, q_mask, k_mask), res


def kernel(q, k, q_mask, k_mask):
    y, _ = run(q, k, q_mask, k_mask, trace=False)
    return y


# revision 11
# speedup vs baseline: 1.4102x; 1.4102x over previous
"""Trainium2 Bass kernel: ColBERT-like scorer.

Computes s[b,o] = LOGIT_SCALE/sqrt(Lq*Lk) * sum_i logsumexp_j(ALPHA * qn[b,i]@kn[o,j]) / ALPHA
with k_mask masking (masked j excluded), q/k L2-normalized along D.

Sharding: O=128 docs split across 8 cores (16 docs each); fully parallel,
concat along axis 1 at the end.

Per-core algorithm (all on device except mask bookkeeping):
  - sumsq via GPSIMD square + DVE tensor_scalar accum; rsqrt(x) =
    exp(-0.5*ln(x)) on ACT (single natural_log_exp table set, forced via an
    explicit LoadActFuncSet so walrus never thrashes table loads).
  - q stays RAW; its norm scale is folded into the exp: per-partition
    activation scale = ALPHA * rsqrt(|q_i|^2).
  - k normalization + mask fold into the PE transpose: kT_n = k_tile.T @
    diag(valid * rsqrt(|k_j|^2)), diag built on GPSIMD from a host identity.
  - main loop: float32r matmul S_raw=[Lq, 2*Lk] into PSUM, ACT
    exp(arq_i * S_raw) over [128,1024] (bf16 out into a per-b [128,16,256]
    tile), then a bf16 pairwise-add tree on DVE (2x mode) + one
    tensor_reduce into sums.
  - subtract n_masked, Ln, ones-matmul to reduce over Lq, final scale.
"""

import os

import numpy as np

import concourse.bacc as bacc
import concourse.bass as bass
import concourse.tile as tile
from concourse import mybir
from concourse.bass_utils import run_bass_kernel_spmd

B, Lq, D = 16, 128, 128
O_FULL, Lk = 128, 256
N_CORES = 8
O_SH = O_FULL // N_CORES          # 16 docs per core
NPAIR = O_SH // 2                 # 8 doc pairs per core
ALPHA = 12.0
LOGIT_SCALE = float(np.exp(-np.log(0.07)))
FINAL_SCALE = LOGIT_SCALE / ALPHA / (float(Lq * Lk) ** 0.5 + 1e-6)
LN_ALPHA = float(np.log(ALPHA))

F32 = mybir.dt.float32
BF16 = mybir.dt.bfloat16
_MM_DT_NAME = os.environ.get("COLBERT_MM_DT", "float32r")
MM_DT = getattr(mybir.dt, _MM_DT_NAME)
# natural_log_exp_and_others in act_info.json — holds both Exp and Ln
ACT_SET_LN_EXP = 6

AF = mybir.ActivationFunctionType
ALU = mybir.AluOpType


def _emit_kernel(tc, q_d, k_d, valid_d, nmb_d, ident_d, y_d):
    nc = tc.nc

    with (
        tc.tile_pool(name="const", bufs=1) as constp,
        tc.tile_pool(name="raw", bufs=1) as rawp,
        tc.tile_pool(name="kn", bufs=NPAIR) as knp,
        tc.tile_pool(name="qn", bufs=4) as qnp,
        tc.tile_pool(name="stat", bufs=1) as statp,
        tc.tile_pool(name="ss", bufs=4) as ssp,
        tc.tile_pool(name="scr", bufs=2) as scrp,
        tc.tile_pool(name="exp", bufs=2) as expp,
        tc.tile_pool(name="ps_prep", bufs=2, space="PSUM") as psprep,
        tc.tile_pool(name="ps_main", bufs=2, space="PSUM") as psmain,
        tc.tile_pool(name="ps_fin", bufs=1, space="PSUM") as psfin,
    ):
        # Pin the ACT table set that contains BOTH Exp and Ln so walrus
        # doesn't alternate between exp_and_others / natural_log sets.
        nc.scalar.add_instruction(
            mybir.InstLoadActFuncSet(
                name=nc.get_next_instruction_name(),
                act_func_set_id=ACT_SET_LN_EXP,
            )
        )

        # ---- constants from host ----
        ident = constp.tile([128, 128], F32, tag="ident", name="ident")
        nc.sync.dma_start(ident[:], ident_d)
        valid = constp.tile([128, 2 * O_SH], F32, tag="valid", name="valid")
        nc.sync.dma_start(valid[:], valid_d)
        nmb = constp.tile([128, B * O_SH], F32, tag="nmb", name="nmb")
        nc.sync.dma_start(nmb[:], nmb_d)
        ones = constp.tile([128, 1], F32, tag="ones", name="ones")
        nc.vector.memset(ones[:], 1.0)
        lnalpha = constp.tile([128, 1], F32, tag="lna", name="lnalpha")
        nc.vector.memset(lnalpha[:], LN_ALPHA)

        # ---- raw input tiles ----
        # kraw[p, t, d] where t = o*2 + h, j = h*128 + p
        kraw = rawp.tile([128, 2 * O_SH, D], F32, tag="kraw", name="kraw")
        k_re = k_d.rearrange("o (h p) d -> p (o h) d", h=2, p=128)
        # qraw[p, b, d] where p = i
        qraw = rawp.tile([128, B, D], F32, tag="qraw", name="qraw")
        q_re = q_d.rearrange("b i d -> i b d")

        sums = statp.tile([128, B * O_SH], F32, tag="sums", name="sums")
        arq = statp.tile([128, B], F32, tag="arq", name="arq")  # ALPHA*rsqrt(|q|^2)

        knT = [None] * NPAIR   # [d, (o2 h p)] = [128, 512] per pair, MM_DT
        qnT = [None] * 4       # [d, (b4 i)] = [128, 512] per group of 4 b, MM_DT

        def prep_q_group(g):
            """Raw-transpose q[b] for b in [4g, 4g+4); alpha*rsqrt norms -> arq."""
            sl = slice(g * 4, (g + 1) * 4)
            nc.sync.dma_start(qraw[:, sl, :], q_re[:, sl, :])
            sq = scrp.tile([128, 4, D], F32, tag="scr", name=f"qsq{g}")
            nc.gpsimd.tensor_tensor(
                out=sq[:], in0=qraw[:, sl, :], in1=qraw[:, sl, :], op=ALU.mult
            )
            ss = ssp.tile([128, 4], F32, tag="ss", name=f"ssq{g}")
            for t in range(4):
                nc.vector.tensor_scalar(
                    out=sq[:, t, :],
                    in0=sq[:, t, :],
                    scalar1=1.0,
                    scalar2=None,
                    op0=ALU.mult,
                    op1=ALU.add,
                    accum_out=ss[:, t : t + 1],
                )
            # arq = ALPHA * ss^-0.5 = exp(-0.5*ln(ss) + ln(ALPHA))
            nc.scalar.activation(ss[:], ss[:], AF.Ln)
            nc.scalar.activation(
                arq[:, sl], ss[:], AF.Exp, scale=-0.5, bias=lnalpha[:]
            )
            ps = psprep.tile([128, 512], F32, tag="tp", name=f"qtp{g}")
            for t in range(4):
                nc.tensor.matmul(
                    ps[:, t * 128 : (t + 1) * 128],
                    qraw[:, g * 4 + t, :],
                    ident[:],
                    is_transpose=True,
                    start=(t == 0),
                    stop=(t == 3),
                )
            sb = qnp.tile([128, 512], MM_DT, tag="qnT", name=f"qnT{g}")
            nc.vector.tensor_copy(sb[:], ps[:])
            qnT[g] = sb

        def prep_k_pair(p):
            """Fused normalize+mask+transpose of k docs 2p, 2p+1."""
            tsl = slice(p * 4, (p + 1) * 4)  # 4 half-tiles
            nc.sync.dma_start(kraw[:, tsl, :], k_re[:, tsl, :])
            sq = scrp.tile([128, 4, D], F32, tag="scr", name=f"ksq{p}")
            nc.gpsimd.tensor_tensor(
                out=sq[:], in0=kraw[:, tsl, :], in1=kraw[:, tsl, :], op=ALU.mult
            )
            ss = ssp.tile([128, 4], F32, tag="ss", name=f"ssk{p}")
            for t in range(4):
                nc.vector.tensor_scalar(
                    out=sq[:, t, :],
                    in0=sq[:, t, :],
                    scalar1=1.0,
                    scalar2=None,
                    op0=ALU.mult,
                    op1=ALU.add,
                    accum_out=ss[:, t : t + 1],
                )
            # rk = ss^-0.5, then fold the k mask in (masked row -> 0)
            nc.scalar.activation(ss[:], ss[:], AF.Ln)
            nc.scalar.activation(ss[:], ss[:], AF.Exp, scale=-0.5)
            nc.vector.tensor_tensor(
                out=ss[:], in0=ss[:], in1=valid[:, tsl], op=ALU.mult
            )
            for t in range(4):
                nc.vector.tensor_scalar(
                    out=kraw[:, p * 4 + t, :],
                    in0=kraw[:, p * 4 + t, :],
                    scalar1=ss[:, t : t + 1],
                    scalar2=None,
                    op0=ALU.mult,
                )
            ps = psprep.tile([128, 512], F32, tag="tp", name=f"ktp{p}")
            for t in range(4):
                nc.tensor.matmul(
                    ps[:, t * 128 : (t + 1) * 128],
                    kraw[:, p * 4 + t, :],
                    ident[:],
                    is_transpose=True,
                    start=(t == 0),
                    stop=(t == 3),
                )
            sb = knp.tile([128, 512], MM_DT, tag="knT", name=f"knT{p}")
            nc.vector.tensor_copy(sb[:], ps[:])
            knT[p] = sb

        # Emit prep: all of k (main loop needs every pair), q interleaved.
        prep_q_group(0)
        for p in range(NPAIR):
            prep_k_pair(p)
            if p % 2 == 1 and p // 2 + 1 < 4:
                prep_q_group(p // 2 + 1)

        # ---- main loop: per b, 4 psum groups of 2 doc-pairs ----
        for b in range(B):
            et = expp.tile([128, O_SH, 2 * Lk // 2], BF16, tag="E", name=f"E{b}")
            # et[p, doc, j'] with j' in [0,256); exp written per group of 4 docs
            for g in range(4):
                ps = psmain.tile([128, 1024], F32, tag="S", name=f"S{b}_{g}")
                for u in range(2):
                    nc.tensor.matmul(
                        ps[:, u * 512 : (u + 1) * 512],
                        qnT[b // 4][:, (b % 4) * 128 : (b % 4 + 1) * 128],
                        knT[g * 2 + u][:],
                        start=True,
                        stop=True,
                    )
                nc.scalar.activation(
                    et[:, g * 4 : (g + 1) * 4, :].rearrange("p a c -> p (a c)"),
                    ps[:],
                    AF.Exp,
                    scale=arq[:, b : b + 1],
                )
            # pairwise-add tree over j' (bf16, 2x mode), then one reduce
            nc.vector.tensor_tensor(
                out=et[:, :, 0:128], in0=et[:, :, 0:128], in1=et[:, :, 128:256],
                op=ALU.add,
            )
            nc.vector.tensor_tensor(
                out=et[:, :, 0:64], in0=et[:, :, 0:64], in1=et[:, :, 64:128],
                op=ALU.add,
            )
            nc.vector.tensor_tensor(
                out=et[:, :, 0:32], in0=et[:, :, 0:32], in1=et[:, :, 32:64],
                op=ALU.add,
            )
            nc.vector.tensor_reduce(
                out=sums[:, b * O_SH : (b + 1) * O_SH],
                in_=et[:, :, 0:32],
                op=ALU.add,
                axis=mybir.AxisListType.X,
            )

        # ---- final: lse = ln(sum - n_masked); s = scale * sum_i lse ----
        sub = statp.tile([128, B * O_SH], F32, tag="sub", name="sub")
        nc.vector.tensor_tensor(out=sub[:], in0=sums[:], in1=nmb[:], op=ALU.subtract)
        nc.scalar.activation(sub[:], sub[:], AF.Ln)
        fin = psfin.tile([1, B * O_SH], F32, tag="fin", name="fin")
        nc.tensor.matmul(fin[:], ones[:], sub[:], start=True, stop=True)
        out_sb = statp.tile([1, B * O_SH], F32, tag="out", name="out_sb")
        nc.scalar.mul(out_sb[:], fin[:], FINAL_SCALE)
        nc.sync.dma_start(y_d, out_sb[:])


def build_nc():
    nc = bacc.Bacc("TRN2", target_bir_lowering=False, debug=False)
    q_d = nc.dram_tensor("q", [B, Lq, D], F32, kind="ExternalInput")
    k_d = nc.dram_tensor("k", [O_SH, Lk, D], F32, kind="ExternalInput")
    valid_d = nc.dram_tensor("valid", [128, 2 * O_SH], F32, kind="ExternalInput")
    nmb_d = nc.dram_tensor("nmb", [128, B * O_SH], F32, kind="ExternalInput")
    ident_d = nc.dram_tensor("ident", [128, 128], F32, kind="ExternalInput")
    y_d = nc.dram_tensor("y", [1, B * O_SH], F32, kind="ExternalOutput")
    with tile.TileContext(nc) as tc:
        _emit_kernel(
            tc, q_d.ap(), k_d.ap(), valid_d.ap(), nmb_d.ap(), ident_d.ap(), y_d.ap()
        )
    nc.compile()
    return nc


_NC = None


def _get_nc():
    global _NC
    if _NC is None:
        _NC = build_nc()
    return _NC


def make_in_maps(q, k, q_mask, k_mask):
    q = np.ascontiguousarray(np.asarray(q, dtype=np.float32))
    k = np.asarray(k, dtype=np.float32)
    k_mask = np.asarray(k_mask, dtype=bool)
    kvalid = (~k_mask).astype(np.float32)               # [O, Lk]
    nmask = k_mask.sum(axis=1).astype(np.float32)       # [O]
    ident = np.eye(128, dtype=np.float32)
    in_maps = []
    for c in range(N_CORES):
        osl = slice(c * O_SH, (c + 1) * O_SH)
        # valid[p, o*2+h] matching kraw tile order
        valid_t = np.ascontiguousarray(
            kvalid[osl].reshape(O_SH, 2, 128).transpose(2, 0, 1).reshape(128, 2 * O_SH)
        )
        nmb_row = np.tile(nmask[osl], B)                # col = b*16 + o
        nmb_t = np.ascontiguousarray(
            np.broadcast_to(nmb_row[None, :], (128, B * O_SH))
        )
        in_maps.append(
            {
                "q": q,
                "k": np.ascontiguousarray(k[osl]),
                "valid": valid_t,
                "nmb": nmb_t,
                "ident": ident,
            }
        )
    return in_maps


def _postprocess(results, q_mask, k_mask):
    y = np.concatenate(
        [results[c]["y"].reshape(B, O_SH) for c in range(N_CORES)], axis=1
    ).astype(np.float32)
    q_mask = np.asarray(q_mask, dtype=bool)
    k_mask = np.asarray(k_mask, dtype=bool)
    # exact emulation of the reference's isfinite -> 0 replacement
    y[q_mask.any(axis=1), :] = 0.0
    y[:, k_mask.all(axis=1)] = 0.0
    return y


def run(q, k, q_mask, k_mask, trace=False, **kwargs):
    nc = _get_nc()
    in_maps = make_in_maps(q, k, q_mask, k_mask)
    res = run_bass_kernel_spmd(
        nc, in_maps, core_ids=list(range(N_CORES)), trace=trace, **kwargs
    )
    return _postprocess(res.results, q_mask, k_mask), res


def kernel(q, k, q_mask, k_mask):
    y, _ = run(q, k, q_mask, k_mask, trace=False)
    return y
